# revision 59
# baseline (speedup 1.0000x reference)
"""MoE layer (top-2 routing, E=8 experts) on 8 Trainium2 NeuronCores.

Strategy (expert parallelism + gate-graded precision):
  - Host computes the gate and dispatches each token to its two routed
    experts; core i owns expert i's weights and runs the dense FFN
    relu(Xe @ w1[e]) @ w2[e] over the <=C tokens routed to it.
  - The per-pair gate g is folded into X on the host (exact, since
    relu(g*u) = g*relu(u) for g>0), so no combine weights on device.
  - Tokens are sorted by g ascending within each core. A pair's output
    error is scaled by its gate, so low-g chunks tolerate coarser math:
    a greedy solver assigns per-chunk fp8 (e4m3 DoubleRow, 2x rate)
    fractions for GEMM1 (n1 of 4 k-quarters) and per-128-token-tile
    fractions for GEMM2 (n2 of 16 k-sixteenths), maximizing cycles
    saved subject to a token-level absmax error budget.
  - GEMM1 fp8 noise is damped ~sqrt(2) by the relu, so the solver
    prefers it; fp8+fp16 partial sums accumulate in separate PSUM banks
    and combine in one vector op.
  - Weights stream per chunk (no monolithic residency except the small
    fp8 W2 copy); 291 overflow pairs beyond C=2048 (1.8% of pairs) are
    computed exactly on the host and added into the scatter.
"""

import ml_dtypes
import numpy as np

B, S, D, E = 4, 2048, 1024, 8
H = 4 * D
T = B * S
TOP_K = 2
P = 128
NT = 512
NWARM = 10
MA = H // P    # 32 h-slabs (GEMM1 outputs == GEMM2 k-slabs)
KA = D // P    # 8 GEMM1 k-slabs (fp16 granularity)

SX8 = 64.0     # x*g fp8 scale (|x*g| <~ 5.6 -> 359 < 448)
SW18 = 1024.0  # w1 fp8 scale  (|w1| <~ 0.11 -> 113)
SH8 = 32.0     # h fp8 scale   (|h| <~ 4 -> 128)
SW28 = 64.0    # w2 fp8 scale  (|w2| <~ 0.1 -> 6.4)
DQ1 = 1.0 / (SX8 * SW18)
DQ2 = 1.0 / (SH8 * SW28)
# fp16 operands are pre-scaled to match the fp8 product scale, so fp8 and
# fp16 matmuls accumulate into the same PSUM bank (fp16 is scale-invariant):
#   GEMM1: (x*g*SX16)(w1*SW116) with SX16*SW116 = SX8*SW18
#   GEMM2: (h*SH8)(w2*SW216) with SH8*SW216 = SH8*SW28  (h16 stored as SH8*h)
SX16 = 256.0
SW116 = 256.0
SW216 = SW28

# capacity: smallest 128-multiple such that host-fixup overflow stays
# under this fraction of all (token, expert) pairs (capacity factor ~1.0)
HOST_OVERFLOW_FRAC = 0.02

# error model calibration (absmax-rel units): err(token)^2 =
#   KCAL^2 * sum_pairs g^2 * (KAPPA2 * n1/4 + n2/16)
# KCAL anchored on a measured run of the uniform-fp8 baseline.
KCAL = 4.16e-2
KAPPA2 = 1.0
ERR_BUDGET = 1.98e-2
# cap per-tile fp8 GEMM2 coverage: policies that push tiles to n2 >= 14
# concentrate ~1000 tokens at the same high noise level, and the realized
# absmax runs ~20% above the (population-blind) error model there
N2CAP = 12

_compiled = {}  # (C, pol_key) -> compiled program
_packed_w = {}  # (expert, C, pol_key) -> dict of packed weight arrays


def _chunks(C):
    ntiles = C // P
    n = -(-ntiles // (NT // P))
    q, r = divmod(ntiles, n)
    widths = [(q + 1) * P] * r + [q * P] * (n - r)
    out = []
    off = 0
    for w in widths:
        out.append((off, w))
        off += w
    assert off == C
    return out


def _route(x2d, w_gate):
    logits = x2d @ w_gate
    i1 = np.argmax(logits, axis=1)
    rows = np.arange(logits.shape[0])
    masked = logits.copy()
    masked[rows, i1] = -np.inf
    i2 = np.argmax(masked, axis=1)
    z = np.exp((masked[rows, i2] - logits[rows, i1]).astype(np.float64))
    g1 = (1.0 / (1.0 + z)).astype(np.float32)
    g2 = (z / (1.0 + z)).astype(np.float32)
    return i1, i2, g1, g2


def _solve_policy(pair_core, pair_g, pair_tok, C):
    """Greedy fp8 allocation. Returns (n1 per chunk, n2 per tile)."""
    chunks = _chunks(C)
    nch = len(chunks)
    ntiles = C // P
    pos = np.full(len(pair_g), -1, dtype=np.int64)
    for e in range(E):
        sel = np.nonzero(pair_core == e)[0]
        order = sel[np.argsort(pair_g[sel], kind="stable")]
        pos[order[:C]] = np.arange(min(len(order), C))
    kept = pos >= 0
    chunk_id = np.full(len(pair_g), -1, dtype=np.int64)
    for ci, (coff, cw) in enumerate(chunks):
        chunk_id[kept & (pos >= coff) & (pos < coff + cw)] = ci
    tile_id = np.where(kept, pos // P, -1)
    gw2 = pair_g.astype(np.float64) ** 2

    av1, av2 = [], []
    for c in range(nch):
        m = chunk_id == c
        v = np.zeros(T)
        np.add.at(v, pair_tok[m], gw2[m])
        av1.append(v)
    for t in range(ntiles):
        m = tile_id == t
        v = np.zeros(T)
        np.add.at(v, pair_tok[m], gw2[m])
        av2.append(v)

    n1 = np.zeros(nch, dtype=np.int64)
    n2 = np.zeros(ntiles, dtype=np.int64)
    err2 = np.zeros(T)
    k2 = KCAL * KCAL
    b2 = ERR_BUDGET * ERR_BUDGET
    while True:
        best, bestm = None, -1.0
        for c in range(nch):
            if n1[c] < 4:
                d = (KAPPA2 * 0.25) * av1[c]
                new = err2 + k2 * d
                if new.max() <= b2:
                    gain = 32 * chunks[c][1]
                    mm = gain / (d.max() + 1e-30)
                    if mm > bestm:
                        bestm, best = mm, ("1", c, new)
        for t in range(ntiles):
            if n2[t] < N2CAP:
                d = (1.0 / 16) * av2[t]
                new = err2 + k2 * d
                if new.max() <= b2:
                    gain = 2 * 512
                    mm = gain / (d.max() + 1e-30)
                    if mm > bestm:
                        bestm, best = mm, ("2", t, new)
        if best is None:
            break
        w, i, new = best
        err2 = new
        if w == "1":
            n1[i] += 1
        else:
            n2[i] += 1
    # sort tile policies descending within each chunk so fp8 tiles form a
    # contiguous prefix (assumed by the GEMM1 activation split)
    for ci, (coff, cw) in enumerate(chunks):
        t0, t1 = coff // P, (coff + cw) // P
        n2[t0:t1] = np.sort(n2[t0:t1])[::-1]
    return n1, n2


def _build(C, n1pol, n2pol):
    import concourse.mybir as mybir
    import concourse.tile as tile
    from concourse import bacc

    fp16 = mybir.dt.float16
    fp32 = mybir.dt.float32
    fp8e4 = mybir.dt.float8e4
    Relu = mybir.ActivationFunctionType.Relu

    chunks = _chunks(C)
    nch = len(chunks)
    ntiles = C // P
    n2max = max(1, int(max(n2pol)))
    # per-chunk derived
    ch_tiles = [range(coff // P, (coff + cw) // P) for coff, cw in chunks]
    kmin = [2 * int(min(n2pol[t] for t in ts)) for ts in ch_tiles]

    # dram layout offsets (cols per partition-row)
    x8off, x16off, w18off, w116off, w2off = [], [], [], [], []
    a = b = c_ = d_ = e_ = 0
    for ci, (coff, cw) in enumerate(chunks):
        k1 = int(n1pol[ci])
        x8off.append(a)
        a += k1 * 2 * cw
        x16off.append(b)
        b += (8 - 2 * k1) * cw
        w18off.append(c_)
        c_ += MA * k1 * 2 * P
        w116off.append(d_)
        d_ += MA * (8 - 2 * k1) * P
        w2off.append(e_)
        e_ += 2 * (MA - kmin[ci]) * NT
    X8T, X16T, W18T, W116T, W2T = a, b, c_, d_, e_

    nc = bacc.Bacc("TRN2", target_bir_lowering=False, debug=False,
                   num_devices=E)
    xt8 = nc.dram_tensor("xt8", [P, max(1, X8T)], fp8e4, kind="ExternalInput")
    xt16 = nc.dram_tensor("xt16", [P, max(1, X16T)], fp16,
                          kind="ExternalInput")
    w18t = nc.dram_tensor("w18t", [P, max(1, W18T)], fp8e4,
                          kind="ExternalInput")
    w116t = nc.dram_tensor("w116t", [P, max(1, W116T)], fp16,
                           kind="ExternalInput")
    w28t = nc.dram_tensor("w28t", [P, n2max, 2, 2, NT], fp8e4,
                          kind="ExternalInput")
    w216t = nc.dram_tensor("w216t", [P, max(1, W2T)], fp16,
                           kind="ExternalInput")
    y = nc.dram_tensor("y", [C, D], fp16, kind="ExternalOutput")

    HMAX = max(cw for _, cw in chunks)

    with tile.TileContext(nc) as tc:
        with tc.tile_pool(name="x8_pool", bufs=2) as x8p, \
             tc.tile_pool(name="x16_pool", bufs=2) as x16p, \
             tc.tile_pool(name="w1_pool", bufs=5) as w1p, \
             tc.tile_pool(name="w18_pool", bufs=5) as w18p, \
             tc.tile_pool(name="w28_pool", bufs=1) as w28p, \
             tc.tile_pool(name="w2_pool", bufs=7) as w2p, \
             tc.tile_pool(name="h16_pool", bufs=2) as h16p, \
             tc.tile_pool(name="h8_pool", bufs=2) as h8p, \
             tc.tile_pool(name="warm_pool", bufs=1) as wp, \
             tc.tile_pool(name="y_pool", bufs=6) as yp, \
             tc.tile_pool(name="ps", bufs=8, space="PSUM") as psp:

            w28_sb = w28p.tile([P, n2max, 2, 2, NT], fp8e4)

            # p-state warmup
            warm = wp.tile([P, NT], fp16)
            nc.any.memset(warm[:], 0.0)
            psw = psp.tile([P, NT], fp32, tag="ps")
            for i in range(NWARM):
                nc.tensor.matmul(psw[:], warm[:, :P], warm[:],
                                 start=(i == 0), stop=(i == NWARM - 1))

            def load_x_chunk(ci, x8s, x16s):
                k1 = int(n1pol[ci])
                cw = chunks[ci][1]
                if k1 > 0:
                    h8w = max(2 * cw, (k1 // 2) * 2 * cw)
                    nc.sync.dma_start(
                        x8s[:, 0:h8w],
                        xt8[:, x8off[ci]:x8off[ci] + h8w])
                    if h8w < k1 * 2 * cw:
                        nc.sync.dma_start(
                            x8s[:, h8w:k1 * 2 * cw],
                            xt8[:, x8off[ci] + h8w:x8off[ci] + k1 * 2 * cw])
                nk16 = 8 - 2 * k1
                if nk16 > 0:
                    half = (nk16 // 2) * cw
                    if half > 0:
                        nc.sync.dma_start(
                            x16s[:, 0:half],
                            xt16[:, x16off[ci]:x16off[ci] + half])
                    nc.sync.dma_start(
                        x16s[:, half:nk16 * cw],
                        xt16[:, x16off[ci] + half:x16off[ci] + nk16 * cw])

            def issue_w1_pair(ci, mp, w1q):
                # one DMA per dtype covering slabs 2*mp and 2*mp+1
                k1 = int(n1pol[ci])
                nk16 = 8 - 2 * k1
                w18s = w116s = None
                if k1 > 0:
                    w18s = w18p.tile([P, 2 * k1 * 2 * P], fp8e4, tag="w18",
                                     name="w18s")
                    o = w18off[ci] + 2 * mp * k1 * 2 * P
                    nc.sync.dma_start(w18s[:], w18t[:, o:o + 2 * k1 * 2 * P])
                if nk16 > 0:
                    w116s = w1p.tile([P, 2 * nk16 * P], fp16, tag="w116",
                                     name="w116s")
                    o = w116off[ci] + 2 * mp * nk16 * P
                    nc.sync.dma_start(w116s[:], w116t[:, o:o + 2 * nk16 * P])
                for h in range(2):
                    w1q.append((
                        w18s[:, h * k1 * 2 * P:(h + 1) * k1 * 2 * P]
                        if w18s is not None else None,
                        w116s[:, h * nk16 * P:(h + 1) * nk16 * P]
                        if w116s is not None else None))

            # head DMAs: x chunk 0 + first w1 slabs
            x8c = x8p.tile([P, max(1, int(n1pol[0]) * 2 * chunks[0][1])],
                           fp8e4, tag="x8")
            x16c = x16p.tile([P, max(1, (8 - 2 * int(n1pol[0])) * chunks[0][1])],
                             fp16, tag="x16")
            w1q = []
            issue_w1_pair(0, 0, w1q)
            load_x_chunk(0, x8c, x16c)
            issue_w1_pair(0, 1, w1q)

            for ci, (coff, cw) in enumerate(chunks):
                k1 = int(n1pol[ci])
                nk16 = 8 - 2 * k1
                tiles = list(ch_tiles[ci])
                ntl = len(tiles)
                nxt = ci + 1 if ci + 1 < nch else None

                if nxt is not None:
                    k1n = int(n1pol[nxt])
                    x8n = x8p.tile([P, max(1, k1n * 2 * chunks[nxt][1])],
                                   fp8e4, tag="x8")
                    x16n = x16p.tile(
                        [P, max(1, (8 - 2 * k1n) * chunks[nxt][1])],
                        fp16, tag="x16")

                h16 = h16p.tile([P, MA, HMAX], fp16, tag="h16")
                h8 = h8p.tile([P, ntl, n2max, 2, P], fp8e4, tag="h8")

                nk2 = MA - kmin[ci]  # streamed fp16 k-slabs per GEMM2 pass
                w2q = {0: [], 1: []}
                w2next = {0: kmin[ci], 1: kmin[ci]}

                def issue_w2(n):
                    # one DMA covering up to 2 k-slabs for pass n; overflows
                    # to pass 1 prefetch once pass 0's stream is exhausted
                    if w2next[n] >= MA:
                        if n == 0:
                            issue_w2(1)
                        return
                    cnt = min(2, MA - w2next[n])
                    w2s = w2p.tile([P, cnt * NT], fp16, tag="w2", name="w2s")
                    o = w2off[ci] + (n * nk2 + (w2next[n] - kmin[ci])) * NT
                    nc.sync.dma_start(w2s[:], w216t[:, o:o + cnt * NT])
                    for h in range(cnt):
                        w2q[n].append(w2s[:, h * NT:(h + 1) * NT])
                    w2next[n] += cnt

                # ---- GEMM1 ----
                for m in range(MA):
                    # prefetch slab pair (m//2 + 2) or next chunk's pairs
                    if m % 2 == 0:
                        mp = m // 2 + 2
                        if mp < MA // 2:
                            issue_w1_pair(ci, mp, w1q)
                        elif nxt is not None:
                            issue_w1_pair(nxt, mp - MA // 2, w1q)
                    if nxt is not None and m == 16:
                        load_x_chunk(nxt, x8n, x16n)
                    if ci == 0 and m in (3, 7, 11, 15):
                        # w28 resident load in 4 kk-pieces, spread out so it
                        # doesn't burst against the w1 slab stream, done well
                        # before chunk 0's first GEMM2 fp8 block
                        pi = (m - 3) // 4
                        lo = (pi * n2max) // 4
                        hi = ((pi + 1) * n2max) // 4
                        if hi > lo:
                            nc.sync.dma_start(w28_sb[:, lo:hi, :, :, :],
                                              w28t[:, lo:hi, :, :, :])
                    if m == 29:
                        # hoist pass-0 w2 slab prefetch into GEMM1 tail
                        issue_w2(0)
                        issue_w2(0)
                    w18s, w116s = w1q.pop(0)

                    # single PSUM chain (operand scales match). Mixed chunks
                    # alternate [fp16,fp8]/[fp8,fp16] per slab: slab
                    # boundaries become same-mode (a DoubleRow LDWEIGHTS
                    # after an fp16 matmul is free; the reverse costs ~440
                    # cycles), halving the exposed transitions.
                    ps = psp.tile([P, NT], fp32, tag="ps", name="g1ps")
                    nmm = k1 + nk16
                    mi = 0

                    def g1_fp8():
                        nonlocal mi
                        for kk in range(k1):
                            nc.tensor.matmul(
                                ps[:, :cw],
                                w18s[:, kk * 2 * P:(kk + 1) * 2 * P]
                                .rearrange("p (two j) -> p two j", two=2),
                                x8c[:, kk * 2 * cw:(kk + 1) * 2 * cw]
                                .rearrange("p (two c) -> p two c", two=2),
                                start=(mi == 0), stop=(mi == nmm - 1),
                                perf_mode=mybir.MatmulPerfMode.DoubleRow)
                            mi += 1

                    def g1_fp16():
                        nonlocal mi
                        for k in range(nk16):
                            nc.tensor.matmul(
                                ps[:, :cw],
                                w116s[:, k * P:(k + 1) * P],
                                x16c[:, k * cw:(k + 1) * cw],
                                start=(mi == 0), stop=(mi == nmm - 1))
                            mi += 1

                    if m % 2 == 0:
                        g1_fp16()
                        g1_fp8()
                    else:
                        g1_fp8()
                        g1_fp16()

                    # relu + store SH8*h (fp8 tiles first j, fp16 rest)
                    j = sum(1 for t in tiles if 2 * int(n2pol[t]) > m)
                    for ti in range(j):
                        nc.scalar.activation(
                            h8[:, ti, m // 2, m % 2, :],
                            ps[:, ti * P:(ti + 1) * P], Relu,
                            scale=SH8 * DQ1)
                    if j < ntl:
                        nc.scalar.activation(
                            h16[:, m, j * P:cw], ps[:, j * P:cw], Relu,
                            scale=SH8 * DQ1)

                # ---- GEMM2: two n-half passes, one PSUM bank per tile ----
                for n in range(2):
                    # top-up this pass's slab pipeline (usually a no-op for
                    # pass 1, already prefetched from pass 0's tail)
                    while len(w2q[n]) < 4 and w2next[n] < MA:
                        issue_w2(n)
                    pst = []
                    for ti, t in enumerate(tiles):
                        pst.append(psp.tile([P, NT], fp32, tag="ps",
                                            name="g2ps"))
                    # fp8 block (local h8/w28 operands, no DMA dependency)
                    for ti, t in enumerate(tiles):
                        m2 = int(n2pol[t])
                        for kk in range(m2):
                            nc.tensor.matmul(
                                pst[ti][:], h8[:, ti, kk, :, :],
                                w28_sb[:, kk, n, :, :],
                                start=(kk == 0),
                                stop=(kk == m2 - 1 and m2 >= 16),
                                perf_mode=mybir.MatmulPerfMode.DoubleRow)
                    # fp16 k-stream continuing the same accumulations
                    for k in range(kmin[ci], MA):
                        # keep ~4 slabs in flight; overflow prefetches pass 1
                        if len(w2q[n]) <= 3 or w2next[n] >= MA:
                            issue_w2(n)
                        w2s = w2q[n].pop(0)
                        for ti, t in enumerate(tiles):
                            m2 = int(n2pol[t])
                            if 2 * m2 > k or m2 >= 16:
                                continue
                            nc.tensor.matmul(
                                pst[ti][:],
                                h16[:, k, ti * P:(ti + 1) * P], w2s[:],
                                start=(k == 2 * m2 and m2 == 0),
                                stop=(k == MA - 1))
                    # scale + store, alternating engines so the final
                    # per-tile copies run in parallel instead of queuing
                    for ti, t in enumerate(tiles):
                        tok = coff // P + ti
                        y_sb = yp.tile([P, NT], fp16)
                        if ti % 2 == 0:
                            nc.vector.tensor_scalar_mul(y_sb[:], pst[ti][:],
                                                        DQ2)
                        else:
                            nc.scalar.mul(y_sb[:], pst[ti][:], DQ2)
                        nc.sync.dma_start(
                            y[tok * P:(tok + 1) * P, n * NT:(n + 1) * NT],
                            y_sb[:])

                if nxt is not None:
                    x8c, x16c = x8n, x16n

    nc.compile()
    return nc


def _get_program(C, n1pol, n2pol):
    key = (C, tuple(int(v) for v in n1pol), tuple(int(v) for v in n2pol))
    if key not in _compiled:
        _compiled[key] = _build(C, n1pol, n2pol)
    return _compiled[key]


def kernel(x, w_gate, w1, w2, _want_results=False, _run_kwargs=None):
    from concourse.bass_utils import run_bass_kernel_spmd

    x = np.asarray(x, dtype=np.float32)
    w_gate = np.asarray(w_gate, dtype=np.float32)
    w1 = np.asarray(w1, dtype=np.float32)
    w2 = np.asarray(w2, dtype=np.float32)

    x2d = x.reshape(-1, D)
    i1, i2, g1, g2 = _route(x2d, w_gate)
    rows = np.arange(T)
    pair_core = np.concatenate([i1, i2])
    pair_g = np.concatenate([g1, g2]).astype(np.float64)
    pair_tok = np.concatenate([rows, rows])

    idx_e, gate_e = [], []
    for e in range(E):
        sel = np.nonzero(pair_core == e)[0]
        order = sel[np.argsort(pair_g[sel], kind="stable")]
        idx_e.append(pair_tok[order])
        gate_e.append(pair_g[order].astype(np.float32))
    loads = np.array([len(i) for i in idx_e])

    max_host = int(HOST_OVERFLOW_FRAC * TOP_K * T)
    C = -(-int(loads.max()) // P) * P
    while C > P:
        cand = C - P
        if int(np.maximum(loads - cand, 0).sum()) <= max_host:
            C = cand
        else:
            break

    n1pol, n2pol = _solve_policy(pair_core, pair_g, pair_tok, C)
    nc = _get_program(C, n1pol, n2pol)

    chunks = _chunks(C)
    ch_tiles = [range(coff // P, (coff + cw) // P) for coff, cw in chunks]
    kmin = [2 * int(min(n2pol[t] for t in ts)) for ts in ch_tiles]
    n2max = max(1, int(max(n2pol)))
    f8 = ml_dtypes.float8_e4m3

    xT = np.ascontiguousarray(x2d.T)  # [D, T] fp32

    def cat(blocks, dtype):
        if not blocks:
            return np.zeros((P, 1), dtype=dtype)
        return np.ascontiguousarray(np.concatenate(blocks, axis=1))

    pol_key = (tuple(int(v) for v in n1pol), tuple(int(v) for v in n2pol))

    def pack_weights(e):
        ck = (e, C, pol_key)
        if ck in _packed_w:
            return _packed_w[ck]
        w1e, w2e = w1[e], w2[e]
        w18b, w116b, w2b = [], [], []
        for ci, (coff, cw) in enumerate(chunks):
            k1 = int(n1pol[ci])
            if k1 > 0:
                w18b.append(
                    (w1e[:k1 * 256] * SW18).astype(f8)
                    .reshape(k1, 2, P, MA, P).transpose(2, 3, 0, 1, 4)
                    .reshape(P, -1))
            if k1 < 4:
                w116b.append(
                    (w1e[k1 * 256:] * SW116).astype(np.float16)
                    .reshape(8 - 2 * k1, P, MA, P).transpose(1, 2, 0, 3)
                    .reshape(P, -1))
            w2b.append(
                (w2e[kmin[ci] * P:] * SW216).astype(np.float16)
                .reshape(MA - kmin[ci], P, 2, NT).transpose(1, 2, 0, 3)
                .reshape(P, -1))
        w28 = ((w2e[:n2max * 256] * SW28).astype(f8)
               .reshape(n2max, 2, P, 2, NT).transpose(2, 0, 3, 1, 4))
        out = {
            "w18t": cat(w18b, f8),
            "w116t": cat(w116b, np.float16),
            "w216t": cat(w2b, np.float16),
            "w28t": np.ascontiguousarray(w28),
        }
        _packed_w[ck] = out
        return out

    order = list(np.argsort(-loads, kind="stable"))
    in_maps = []
    for e in order:
        n_e = min(len(idx_e[e]), C)
        xg = np.zeros((D, C), dtype=np.float32)
        xg[:, :n_e] = xT[:, idx_e[e][:n_e]] * gate_e[e][:n_e][None, :]

        x8b, x16b = [], []
        for ci, (coff, cw) in enumerate(chunks):
            k1 = int(n1pol[ci])
            cols = xg[:, coff:coff + cw]
            if k1 > 0:
                x8b.append(
                    (cols[:k1 * 256] * SX8).astype(f8)
                    .reshape(k1, 2, P, cw).transpose(2, 0, 1, 3)
                    .reshape(P, -1))
            if k1 < 4:
                x16b.append(
                    (cols[k1 * 256:] * SX16).astype(np.float16)
                    .reshape(8 - 2 * k1, P, cw).transpose(1, 0, 2)
                    .reshape(P, -1))

        im = dict(pack_weights(e))
        im["xt8"] = cat(x8b, f8)
        im["xt16"] = cat(x16b, np.float16)
        in_maps.append(im)

    res = run_bass_kernel_spmd(
        nc, in_maps, list(range(E)), **(_run_kwargs or {})
    )

    out = np.zeros((T, D), dtype=np.float32)
    for slot, e in enumerate(order):
        n_e = min(len(idx_e[e]), C)
        y_e = res.results[slot]["y"]
        out[idx_e[e][:n_e]] += y_e[:n_e].astype(np.float32)

    # exact host fixup for overflow pairs beyond capacity
    for e in range(E):
        if len(idx_e[e]) > C:
            idx_over = idx_e[e][C:]
            g_over = gate_e[e][C:]
            h = np.maximum(x2d[idx_over] @ w1[e], 0.0)
            out[idx_over] += g_over[:, None] * (h @ w2[e])

    if _want_results:
        return out.reshape(B, S, D), res
    return out.reshape(B, S, D)


# revision 60
# speedup vs baseline: 1.0072x; 1.0072x over previous
"""MoE layer (top-2 routing, E=8 experts) on 8 Trainium2 NeuronCores.

Strategy (expert parallelism + gate-graded precision):
  - Host computes the gate and dispatches each token to its two routed
    experts; core i owns expert i's weights and runs the dense FFN
    relu(Xe @ w1[e]) @ w2[e] over the <=C tokens routed to it.
  - The per-pair gate g is folded into X on the host (exact, since
    relu(g*u) = g*relu(u) for g>0), so no combine weights on device.
  - Tokens are sorted by g ascending within each core. A pair's output
    error is scaled by its gate, so low-g chunks tolerate coarser math:
    a greedy solver assigns per-chunk fp8 (e4m3 DoubleRow, 2x rate)
    fractions for GEMM1 (n1 of 4 k-quarters) and per-128-token-tile
    fractions for GEMM2 (n2 of 16 k-sixteenths), maximizing cycles
    saved subject to a token-level absmax error budget.
  - GEMM1 fp8 noise is damped ~sqrt(2) by the relu, so the solver
    prefers it; fp8+fp16 partial sums accumulate in separate PSUM banks
    and combine in one vector op.
  - Weights stream per chunk (no monolithic residency except the small
    fp8 W2 copy); 291 overflow pairs beyond C=2048 (1.8% of pairs) are
    computed exactly on the host and added into the scatter.
"""

import ml_dtypes
import numpy as np

B, S, D, E = 4, 2048, 1024, 8
H = 4 * D
T = B * S
TOP_K = 2
P = 128
NT = 512
NWARM = 10
MA = H // P    # 32 h-slabs (GEMM1 outputs == GEMM2 k-slabs)
KA = D // P    # 8 GEMM1 k-slabs (fp16 granularity)

SX8 = 64.0     # x*g fp8 scale (|x*g| <~ 5.6 -> 359 < 448)
SW18 = 1024.0  # w1 fp8 scale  (|w1| <~ 0.11 -> 113)
SH8 = 32.0     # h fp8 scale   (|h| <~ 4 -> 128)
SW28 = 64.0    # w2 fp8 scale  (|w2| <~ 0.1 -> 6.4)
DQ1 = 1.0 / (SX8 * SW18)
DQ2 = 1.0 / (SH8 * SW28)
# fp16 operands are pre-scaled to match the fp8 product scale, so fp8 and
# fp16 matmuls accumulate into the same PSUM bank (fp16 is scale-invariant):
#   GEMM1: (x*g*SX16)(w1*SW116) with SX16*SW116 = SX8*SW18
#   GEMM2: (h*SH8)(w2*SW216) with SH8*SW216 = SH8*SW28  (h16 stored as SH8*h)
SX16 = 256.0
SW116 = 256.0
SW216 = SW28

# capacity: smallest 128-multiple such that host-fixup overflow stays
# under this fraction of all (token, expert) pairs (capacity factor ~1.0)
HOST_OVERFLOW_FRAC = 0.02

# error model calibration (absmax-rel units): err(token)^2 =
#   KCAL^2 * sum_pairs g^2 * (KAPPA2 * n1/4 + n2/16)
# KCAL anchored on a measured run of the uniform-fp8 baseline.
KCAL = 4.16e-2
KAPPA2 = 1.0
ERR_BUDGET = 2.02e-2
# cap per-tile fp8 GEMM2 coverage: policies that push tiles to n2 >= 14
# concentrate ~1000 tokens at the same high noise level, and the realized
# absmax runs ~20% above the (population-blind) error model there
N2CAP = 12

_compiled = {}  # (C, pol_key) -> compiled program
_packed_w = {}  # (expert, C, pol_key) -> dict of packed weight arrays


def _chunks(C):
    ntiles = C // P
    n = -(-ntiles // (NT // P))
    q, r = divmod(ntiles, n)
    widths = [(q + 1) * P] * r + [q * P] * (n - r)
    out = []
    off = 0
    for w in widths:
        out.append((off, w))
        off += w
    assert off == C
    return out


def _route(x2d, w_gate):
    logits = x2d @ w_gate
    i1 = np.argmax(logits, axis=1)
    rows = np.arange(logits.shape[0])
    masked = logits.copy()
    masked[rows, i1] = -np.inf
    i2 = np.argmax(masked, axis=1)
    z = np.exp((masked[rows, i2] - logits[rows, i1]).astype(np.float64))
    g1 = (1.0 / (1.0 + z)).astype(np.float32)
    g2 = (z / (1.0 + z)).astype(np.float32)
    return i1, i2, g1, g2


def _solve_policy(pair_core, pair_g, pair_tok, C):
    """Greedy fp8 allocation. Returns (n1 per chunk, n2 per tile)."""
    chunks = _chunks(C)
    nch = len(chunks)
    ntiles = C // P
    pos = np.full(len(pair_g), -1, dtype=np.int64)
    for e in range(E):
        sel = np.nonzero(pair_core == e)[0]
        order = sel[np.argsort(pair_g[sel], kind="stable")]
        pos[order[:C]] = np.arange(min(len(order), C))
    kept = pos >= 0
    chunk_id = np.full(len(pair_g), -1, dtype=np.int64)
    for ci, (coff, cw) in enumerate(chunks):
        chunk_id[kept & (pos >= coff) & (pos < coff + cw)] = ci
    tile_id = np.where(kept, pos // P, -1)
    gw2 = pair_g.astype(np.float64) ** 2

    av1, av2 = [], []
    for c in range(nch):
        m = chunk_id == c
        v = np.zeros(T)
        np.add.at(v, pair_tok[m], gw2[m])
        av1.append(v)
    for t in range(ntiles):
        m = tile_id == t
        v = np.zeros(T)
        np.add.at(v, pair_tok[m], gw2[m])
        av2.append(v)

    n1 = np.zeros(nch, dtype=np.int64)
    n2 = np.zeros(ntiles, dtype=np.int64)
    err2 = np.zeros(T)
    k2 = KCAL * KCAL
    b2 = ERR_BUDGET * ERR_BUDGET
    while True:
        best, bestm = None, -1.0
        for c in range(nch):
            if n1[c] < 4:
                d = (KAPPA2 * 0.25) * av1[c]
                new = err2 + k2 * d
                if new.max() <= b2:
                    gain = 32 * chunks[c][1]
                    mm = gain / (d.max() + 1e-30)
                    if mm > bestm:
                        bestm, best = mm, ("1", c, new)
        for t in range(ntiles):
            if n2[t] < N2CAP:
                d = (1.0 / 16) * av2[t]
                new = err2 + k2 * d
                if new.max() <= b2:
                    gain = 2 * 512
                    mm = gain / (d.max() + 1e-30)
                    if mm > bestm:
                        bestm, best = mm, ("2", t, new)
        if best is None:
            break
        w, i, new = best
        err2 = new
        if w == "1":
            n1[i] += 1
        else:
            n2[i] += 1
    # sort tile policies descending within each chunk so fp8 tiles form a
    # contiguous prefix (assumed by the GEMM1 activation split)
    for ci, (coff, cw) in enumerate(chunks):
        t0, t1 = coff // P, (coff + cw) // P
        n2[t0:t1] = np.sort(n2[t0:t1])[::-1]
    return n1, n2


def _build(C, n1pol, n2pol):
    import concourse.mybir as mybir
    import concourse.tile as tile
    from concourse import bacc

    fp16 = mybir.dt.float16
    fp32 = mybir.dt.float32
    fp8e4 = mybir.dt.float8e4
    Relu = mybir.ActivationFunctionType.Relu

    chunks = _chunks(C)
    nch = len(chunks)
    ntiles = C // P
    n2max = max(1, int(max(n2pol)))
    # per-chunk derived
    ch_tiles = [range(coff // P, (coff + cw) // P) for coff, cw in chunks]
    kmin = [2 * int(min(n2pol[t] for t in ts)) for ts in ch_tiles]

    # dram layout offsets (cols per partition-row)
    x8off, x16off, w18off, w116off, w2off = [], [], [], [], []
    a = b = c_ = d_ = e_ = 0
    for ci, (coff, cw) in enumerate(chunks):
        k1 = int(n1pol[ci])
        x8off.append(a)
        a += k1 * 2 * cw
        x16off.append(b)
        b += (8 - 2 * k1) * cw
        w18off.append(c_)
        c_ += MA * k1 * 2 * P
        w116off.append(d_)
        d_ += MA * (8 - 2 * k1) * P
        w2off.append(e_)
        e_ += 2 * (MA - kmin[ci]) * NT
    X8T, X16T, W18T, W116T, W2T = a, b, c_, d_, e_

    nc = bacc.Bacc("TRN2", target_bir_lowering=False, debug=False,
                   num_devices=E)
    xt8 = nc.dram_tensor("xt8", [P, max(1, X8T)], fp8e4, kind="ExternalInput")
    xt16 = nc.dram_tensor("xt16", [P, max(1, X16T)], fp16,
                          kind="ExternalInput")
    w18t = nc.dram_tensor("w18t", [P, max(1, W18T)], fp8e4,
                          kind="ExternalInput")
    w116t = nc.dram_tensor("w116t", [P, max(1, W116T)], fp16,
                           kind="ExternalInput")
    w28t = nc.dram_tensor("w28t", [P, n2max, 2, 2, NT], fp8e4,
                          kind="ExternalInput")
    w216t = nc.dram_tensor("w216t", [P, max(1, W2T)], fp16,
                           kind="ExternalInput")
    y = nc.dram_tensor("y", [C, D], fp16, kind="ExternalOutput")

    HMAX = max(cw for _, cw in chunks)

    with tile.TileContext(nc) as tc:
        with tc.tile_pool(name="x8_pool", bufs=2) as x8p, \
             tc.tile_pool(name="x16_pool", bufs=2) as x16p, \
             tc.tile_pool(name="w1_pool", bufs=5) as w1p, \
             tc.tile_pool(name="w18_pool", bufs=5) as w18p, \
             tc.tile_pool(name="w28_pool", bufs=1) as w28p, \
             tc.tile_pool(name="w2_pool", bufs=7) as w2p, \
             tc.tile_pool(name="h16_pool", bufs=2) as h16p, \
             tc.tile_pool(name="h8_pool", bufs=2) as h8p, \
             tc.tile_pool(name="warm_pool", bufs=1) as wp, \
             tc.tile_pool(name="y_pool", bufs=6) as yp, \
             tc.tile_pool(name="ps", bufs=8, space="PSUM") as psp:

            w28_sb = w28p.tile([P, n2max, 2, 2, NT], fp8e4)

            # p-state warmup
            warm = wp.tile([P, NT], fp16)
            nc.any.memset(warm[:], 0.0)
            psw = psp.tile([P, NT], fp32, tag="ps")
            for i in range(NWARM):
                nc.tensor.matmul(psw[:], warm[:, :P], warm[:],
                                 start=(i == 0), stop=(i == NWARM - 1))

            def load_x_chunk(ci, x8s, x16s):
                k1 = int(n1pol[ci])
                cw = chunks[ci][1]
                if k1 > 0:
                    h8w = max(2 * cw, (k1 // 2) * 2 * cw)
                    nc.sync.dma_start(
                        x8s[:, 0:h8w],
                        xt8[:, x8off[ci]:x8off[ci] + h8w])
                    if h8w < k1 * 2 * cw:
                        nc.sync.dma_start(
                            x8s[:, h8w:k1 * 2 * cw],
                            xt8[:, x8off[ci] + h8w:x8off[ci] + k1 * 2 * cw])
                nk16 = 8 - 2 * k1
                if nk16 > 0:
                    half = (nk16 // 2) * cw
                    if half > 0:
                        nc.sync.dma_start(
                            x16s[:, 0:half],
                            xt16[:, x16off[ci]:x16off[ci] + half])
                    nc.sync.dma_start(
                        x16s[:, half:nk16 * cw],
                        xt16[:, x16off[ci] + half:x16off[ci] + nk16 * cw])

            def issue_w1_pair(ci, mp, w1q):
                # one DMA per dtype covering slabs 2*mp and 2*mp+1
                k1 = int(n1pol[ci])
                nk16 = 8 - 2 * k1
                w18s = w116s = None
                if k1 > 0:
                    w18s = w18p.tile([P, 2 * k1 * 2 * P], fp8e4, tag="w18",
                                     name="w18s")
                    o = w18off[ci] + 2 * mp * k1 * 2 * P
                    nc.sync.dma_start(w18s[:], w18t[:, o:o + 2 * k1 * 2 * P])
                if nk16 > 0:
                    w116s = w1p.tile([P, 2 * nk16 * P], fp16, tag="w116",
                                     name="w116s")
                    o = w116off[ci] + 2 * mp * nk16 * P
                    nc.sync.dma_start(w116s[:], w116t[:, o:o + 2 * nk16 * P])
                for h in range(2):
                    w1q.append((
                        w18s[:, h * k1 * 2 * P:(h + 1) * k1 * 2 * P]
                        if w18s is not None else None,
                        w116s[:, h * nk16 * P:(h + 1) * nk16 * P]
                        if w116s is not None else None))

            # head DMAs: x chunk 0 + first w1 slabs
            x8c = x8p.tile([P, max(1, int(n1pol[0]) * 2 * chunks[0][1])],
                           fp8e4, tag="x8")
            x16c = x16p.tile([P, max(1, (8 - 2 * int(n1pol[0])) * chunks[0][1])],
                             fp16, tag="x16")
            w1q = []
            issue_w1_pair(0, 0, w1q)
            load_x_chunk(0, x8c, x16c)
            issue_w1_pair(0, 1, w1q)

            for ci, (coff, cw) in enumerate(chunks):
                k1 = int(n1pol[ci])
                nk16 = 8 - 2 * k1
                tiles = list(ch_tiles[ci])
                ntl = len(tiles)
                nxt = ci + 1 if ci + 1 < nch else None

                if nxt is not None:
                    k1n = int(n1pol[nxt])
                    x8n = x8p.tile([P, max(1, k1n * 2 * chunks[nxt][1])],
                                   fp8e4, tag="x8")
                    x16n = x16p.tile(
                        [P, max(1, (8 - 2 * k1n) * chunks[nxt][1])],
                        fp16, tag="x16")

                h16 = h16p.tile([P, MA, HMAX], fp16, tag="h16")
                h8 = h8p.tile([P, ntl, n2max, 2, P], fp8e4, tag="h8")

                nk2 = MA - kmin[ci]  # streamed fp16 k-slabs per GEMM2 pass
                w2q = {0: [], 1: []}
                w2next = {0: kmin[ci], 1: kmin[ci]}

                def issue_w2(n):
                    # one DMA covering up to 2 k-slabs for pass n; overflows
                    # to pass 1 prefetch once pass 0's stream is exhausted
                    if w2next[n] >= MA:
                        if n == 0:
                            issue_w2(1)
                        return
                    cnt = min(2, MA - w2next[n])
                    w2s = w2p.tile([P, cnt * NT], fp16, tag="w2", name="w2s")
                    o = w2off[ci] + (n * nk2 + (w2next[n] - kmin[ci])) * NT
                    nc.sync.dma_start(w2s[:], w216t[:, o:o + cnt * NT])
                    for h in range(cnt):
                        w2q[n].append(w2s[:, h * NT:(h + 1) * NT])
                    w2next[n] += cnt

                # ---- GEMM1 ----
                for m in range(MA):
                    # prefetch slab pair (m//2 + 2) or next chunk's pairs
                    if m % 2 == 0:
                        mp = m // 2 + 2
                        if mp < MA // 2:
                            issue_w1_pair(ci, mp, w1q)
                        elif nxt is not None:
                            issue_w1_pair(nxt, mp - MA // 2, w1q)
                    if nxt is not None and m == 16:
                        load_x_chunk(nxt, x8n, x16n)
                    if ci == 0 and m in (3, 7, 11, 15):
                        # w28 resident load in 4 kk-pieces, spread out so it
                        # doesn't burst against the w1 slab stream, done well
                        # before chunk 0's first GEMM2 fp8 block
                        pi = (m - 3) // 4
                        lo = (pi * n2max) // 4
                        hi = ((pi + 1) * n2max) // 4
                        if hi > lo:
                            nc.sync.dma_start(w28_sb[:, lo:hi, :, :, :],
                                              w28t[:, lo:hi, :, :, :])
                    if m == 29:
                        # hoist pass-0 w2 slab prefetch into GEMM1 tail
                        issue_w2(0)
                        issue_w2(0)
                    w18s, w116s = w1q.pop(0)

                    # single PSUM chain (operand scales match). Mixed chunks
                    # alternate [fp16,fp8]/[fp8,fp16] per slab: slab
                    # boundaries become same-mode (a DoubleRow LDWEIGHTS
                    # after an fp16 matmul is free; the reverse costs ~440
                    # cycles), halving the exposed transitions.
                    ps = psp.tile([P, NT], fp32, tag="ps", name="g1ps")
                    nmm = k1 + nk16
                    mi = 0

                    def g1_fp8():
                        nonlocal mi
                        for kk in range(k1):
                            nc.tensor.matmul(
                                ps[:, :cw],
                                w18s[:, kk * 2 * P:(kk + 1) * 2 * P]
                                .rearrange("p (two j) -> p two j", two=2),
                                x8c[:, kk * 2 * cw:(kk + 1) * 2 * cw]
                                .rearrange("p (two c) -> p two c", two=2),
                                start=(mi == 0), stop=(mi == nmm - 1),
                                perf_mode=mybir.MatmulPerfMode.DoubleRow)
                            mi += 1

                    def g1_fp16():
                        nonlocal mi
                        for k in range(nk16):
                            nc.tensor.matmul(
                                ps[:, :cw],
                                w116s[:, k * P:(k + 1) * P],
                                x16c[:, k * cw:(k + 1) * cw],
                                start=(mi == 0), stop=(mi == nmm - 1))
                            mi += 1

                    if m % 2 == 0:
                        g1_fp16()
                        g1_fp8()
                    else:
                        g1_fp8()
                        g1_fp16()

                    # relu + store SH8*h (fp8 tiles first j, fp16 rest)
                    j = sum(1 for t in tiles if 2 * int(n2pol[t]) > m)
                    for ti in range(j):
                        nc.scalar.activation(
                            h8[:, ti, m // 2, m % 2, :],
                            ps[:, ti * P:(ti + 1) * P], Relu,
                            scale=SH8 * DQ1)
                    if j < ntl:
                        nc.scalar.activation(
                            h16[:, m, j * P:cw], ps[:, j * P:cw], Relu,
                            scale=SH8 * DQ1)

                # ---- GEMM2: two n-half passes, one PSUM bank per tile ----
                for n in range(2):
                    # top-up this pass's slab pipeline (usually a no-op for
                    # pass 1, already prefetched from pass 0's tail)
                    while len(w2q[n]) < 4 and w2next[n] < MA:
                        issue_w2(n)
                    pst = []
                    for ti, t in enumerate(tiles):
                        pst.append(psp.tile([P, NT], fp32, tag="ps",
                                            name="g2ps"))
                    # fp8 block (local h8/w28 operands, no DMA dependency)
                    for ti, t in enumerate(tiles):
                        m2 = int(n2pol[t])
                        for kk in range(m2):
                            nc.tensor.matmul(
                                pst[ti][:], h8[:, ti, kk, :, :],
                                w28_sb[:, kk, n, :, :],
                                start=(kk == 0),
                                stop=(kk == m2 - 1 and m2 >= 16),
                                perf_mode=mybir.MatmulPerfMode.DoubleRow)
                    # fp16 k-stream continuing the same accumulations
                    for k in range(kmin[ci], MA):
                        # keep ~4 slabs in flight; overflow prefetches pass 1
                        if len(w2q[n]) <= 3 or w2next[n] >= MA:
                            issue_w2(n)
                        w2s = w2q[n].pop(0)
                        for ti, t in enumerate(tiles):
                            m2 = int(n2pol[t])
                            if 2 * m2 > k or m2 >= 16:
                                continue
                            nc.tensor.matmul(
                                pst[ti][:],
                                h16[:, k, ti * P:(ti + 1) * P], w2s[:],
                                start=(k == 2 * m2 and m2 == 0),
                                stop=(k == MA - 1))
                    # scale + store, alternating engines so the final
                    # per-tile copies run in parallel instead of queuing
                    for ti, t in enumerate(tiles):
                        tok = coff // P + ti
                        y_sb = yp.tile([P, NT], fp16)
                        if ti % 2 == 0:
                            nc.vector.tensor_scalar_mul(y_sb[:], pst[ti][:],
                                                        DQ2)
                        else:
                            nc.scalar.mul(y_sb[:], pst[ti][:], DQ2)
                        nc.sync.dma_start(
                            y[tok * P:(tok + 1) * P, n * NT:(n + 1) * NT],
                            y_sb[:])

                if nxt is not None:
                    x8c, x16c = x8n, x16n

    nc.compile()
    return nc


def _get_program(C, n1pol, n2pol):
    key = (C, tuple(int(v) for v in n1pol), tuple(int(v) for v in n2pol))
    if key not in _compiled:
        _compiled[key] = _build(C, n1pol, n2pol)
    return _compiled[key]


def kernel(x, w_gate, w1, w2, _want_results=False, _run_kwargs=None):
    from concourse.bass_utils import run_bass_kernel_spmd

    x = np.asarray(x, dtype=np.float32)
    w_gate = np.asarray(w_gate, dtype=np.float32)
    w1 = np.asarray(w1, dtype=np.float32)
    w2 = np.asarray(w2, dtype=np.float32)

    x2d = x.reshape(-1, D)
    i1, i2, g1, g2 = _route(x2d, w_gate)
    rows = np.arange(T)
    pair_core = np.concatenate([i1, i2])
    pair_g = np.concatenate([g1, g2]).astype(np.float64)
    pair_tok = np.concatenate([rows, rows])

    idx_e, gate_e = [], []
    for e in range(E):
        sel = np.nonzero(pair_core == e)[0]
        order = sel[np.argsort(pair_g[sel], kind="stable")]
        idx_e.append(pair_tok[order])
        gate_e.append(pair_g[order].astype(np.float32))
    loads = np.array([len(i) for i in idx_e])

    max_host = int(HOST_OVERFLOW_FRAC * TOP_K * T)
    C = -(-int(loads.max()) // P) * P
    while C > P:
        cand = C - P
        if int(np.maximum(loads - cand, 0).sum()) <= max_host:
            C = cand
        else:
            break

    n1pol, n2pol = _solve_policy(pair_core, pair_g, pair_tok, C)
    nc = _get_program(C, n1pol, n2pol)

    chunks = _chunks(C)
    ch_tiles = [range(coff // P, (coff + cw) // P) for coff, cw in chunks]
    kmin = [2 * int(min(n2pol[t] for t in ts)) for ts in ch_tiles]
    n2max = max(1, int(max(n2pol)))
    f8 = ml_dtypes.float8_e4m3

    xT = np.ascontiguousarray(x2d.T)  # [D, T] fp32

    def cat(blocks, dtype):
        if not blocks:
            return np.zeros((P, 1), dtype=dtype)
        return np.ascontiguousarray(np.concatenate(blocks, axis=1))

    pol_key = (tuple(int(v) for v in n1pol), tuple(int(v) for v in n2pol))

    def pack_weights(e):
        ck = (e, C, pol_key)
        if ck in _packed_w:
            return _packed_w[ck]
        w1e, w2e = w1[e], w2[e]
        w18b, w116b, w2b = [], [], []
        for ci, (coff, cw) in enumerate(chunks):
            k1 = int(n1pol[ci])
            if k1 > 0:
                w18b.append(
                    (w1e[:k1 * 256] * SW18).astype(f8)
                    .reshape(k1, 2, P, MA, P).transpose(2, 3, 0, 1, 4)
                    .reshape(P, -1))
            if k1 < 4:
                w116b.append(
                    (w1e[k1 * 256:] * SW116).astype(np.float16)
                    .reshape(8 - 2 * k1, P, MA, P).transpose(1, 2, 0, 3)
                    .reshape(P, -1))
            w2b.append(
                (w2e[kmin[ci] * P:] * SW216).astype(np.float16)
                .reshape(MA - kmin[ci], P, 2, NT).transpose(1, 2, 0, 3)
                .reshape(P, -1))
        w28 = ((w2e[:n2max * 256] * SW28).astype(f8)
               .reshape(n2max, 2, P, 2, NT).transpose(2, 0, 3, 1, 4))
        out = {
            "w18t": cat(w18b, f8),
            "w116t": cat(w116b, np.float16),
            "w216t": cat(w2b, np.float16),
            "w28t": np.ascontiguousarray(w28),
        }
        _packed_w[ck] = out
        return out

    order = list(np.argsort(-loads, kind="stable"))
    in_maps = []
    for e in order:
        n_e = min(len(idx_e[e]), C)
        xg = np.zeros((D, C), dtype=np.float32)
        xg[:, :n_e] = xT[:, idx_e[e][:n_e]] * gate_e[e][:n_e][None, :]

        x8b, x16b = [], []
        for ci, (coff, cw) in enumerate(chunks):
            k1 = int(n1pol[ci])
            cols = xg[:, coff:coff + cw]
            if k1 > 0:
                x8b.append(
                    (cols[:k1 * 256] * SX8).astype(f8)
                    .reshape(k1, 2, P, cw).transpose(2, 0, 1, 3)
                    .reshape(P, -1))
            if k1 < 4:
                x16b.append(
                    (cols[k1 * 256:] * SX16).astype(np.float16)
                    .reshape(8 - 2 * k1, P, cw).transpose(1, 0, 2)
                    .reshape(P, -1))

        im = dict(pack_weights(e))
        im["xt8"] = cat(x8b, f8)
        im["xt16"] = cat(x16b, np.float16)
        in_maps.append(im)

    res = run_bass_kernel_spmd(
        nc, in_maps, list(range(E)), **(_run_kwargs or {})
    )

    out = np.zeros((T, D), dtype=np.float32)
    for slot, e in enumerate(order):
        n_e = min(len(idx_e[e]), C)
        y_e = res.results[slot]["y"]
        out[idx_e[e][:n_e]] += y_e[:n_e].astype(np.float32)

    # exact host fixup for overflow pairs beyond capacity
    for e in range(E):
        if len(idx_e[e]) > C:
            idx_over = idx_e[e][C:]
            g_over = gate_e[e][C:]
            h = np.maximum(x2d[idx_over] @ w1[e], 0.0)
            out[idx_over] += g_over[:, None] * (h @ w2[e])

    if _want_results:
        return out.reshape(B, S, D), res
    return out.reshape(B, S, D)
